# revision 23
# baseline (speedup 1.0000x reference)
"""Trainium2 Bass kernel: ESM self-attention (B=4, S=1024, H=1280, NH=20, HD=64).

Sharding: 8 cores = 4 batches x 2 head-groups (10 heads each core).
Host pre-work (layout only): transpose hidden/weights, fold the 1/sqrt(HD)
q-scale into Wq, precompute rotary cos/sin tables (sign folded into sin).

Device per core (all matmuls bf16 — fp8 DoubleRow was tried and fails the
2e-2 tolerance; see session notes):
  qT/kT = WT.T @ hT                (head-pair tiles [128, S])
  PSUM evac -> bf16, rotary via 4 partition-shift DMAs + 3 DVE 4x-mode ops
  scoresT[k,q] = kT.T @ qT         (K=64 contraction)
  probsT = exp(scoresT) on ACT     (bf16 out; ACT does nothing else)
  ctx[q,d], denom[q] = probsT.T @ [v | ones]  (4 q-tiles packed per PSUM bank
    via one-start accumulation; norm = 1 recip + 1 mul per bank on DVE)
  out stored bf16, one DMA per head.
Host: concatenate per-core [S, 640] shards into [B, S, 1280].
"""
import os
import sys

sys.path.insert(0, '/opt/trn_rl_repo')

import numpy as np
import ml_dtypes

B, S, H = 4, 1024, 1280
NH, HD = 20, 64
P = 128
NKT = S // P      # 8 seq tiles
NHT = H // P      # 10 hidden tiles
NHC = NH // 2     # 10 heads per core
NPAIR = NHC // 2  # 5 head pairs per core
GW = NHC * HD     # 640 output columns per core
N_CORES = 8

_cache = {}


def _build(loop_reps=1, ablate=()):
    ablate = set(ablate)
    from contextlib import nullcontext
    from concourse import bacc, tile, mybir

    f32 = mybir.dt.float32
    bf16 = mybir.dt.bfloat16
    Exp = mybir.ActivationFunctionType.Exp

    nc = bacc.Bacc("TRN2", target_bir_lowering=False, debug=False,
                   enable_asserts=True, num_devices=N_CORES)

    hT = nc.dram_tensor("hT", [H, S], bf16, kind="ExternalInput").ap()
    wqT = nc.dram_tensor("wqT", [H, GW], bf16, kind="ExternalInput").ap()
    wkT = nc.dram_tensor("wkT", [H, GW], bf16, kind="ExternalInput").ap()
    wvT = nc.dram_tensor("wvT", [H, GW], bf16, kind="ExternalInput").ap()
    rotb = nc.dram_tensor("rotb", [P, 2 * S], bf16, kind="ExternalInput").ap()
    out = nc.dram_tensor("out", [S, GW], bf16, kind="ExternalOutput").ap()

    # Partition-major views: slab i of hT is rows [i*128,(i+1)*128) -> p i c
    hT_r = hT.rearrange("(i p) c -> p i c", p=P)
    w_r = {"q": wqT.rearrange("(i p) c -> p i c", p=P),
           "k": wkT.rearrange("(i p) c -> p i c", p=P),
           "v": wvT.rearrange("(i p) c -> p i c", p=P)}

    with tile.TileContext(nc) as tc, \
         tc.tile_pool(name="const", bufs=1) as cpool, \
         tc.tile_pool(name="st", bufs=2) as stpool, \
         tc.tile_pool(name="qk", bufs=2) as qkpool, \
         tc.tile_pool(name="probs", bufs=2) as ppool, \
         tc.tile_pool(name="osb", bufs=2) as opool, \
         tc.tile_pool(name="psp", bufs=2, space="PSUM") as pspool, \
         tc.tile_pool(name="pss", bufs=2, space="PSUM") as sspool, \
         tc.tile_pool(name="psc", bufs=2, space="PSUM") as scpool, \
         (tc.For_i(0, loop_reps, 1) if loop_reps > 1 else nullcontext()):

        # Dependency tracking is tile-granular, so every independently-gating
        # input gets its own tile: h as 5 two-slab tiles, q/k weights split
        # into the pair-0 column block (loaded first, 320KB) + the rest.
        # DMA transfers serialize at ~360GB/s, so order = scores(0) critical
        # path first: wq-j0, wk-j0, rot, h; then wq-rest, wk-rest, wv.
        skip_in = "indma" in ablate
        rot_t = cpool.tile([P, 2 * S], bf16, tag="rot")
        cos_t, ssin_t = rot_t[:, 0:S], rot_t[:, S:2 * S]
        hsb = [cpool.tile([P, 2 * S], bf16, tag=f"h{i2}", name=f"h{i2}")
               for i2 in range(NHT // 2)]
        wj0 = {nm: cpool.tile([P, NHT * P], bf16, tag=f"wj0{nm}", name=f"wj0{nm}")
               for nm in ("q", "k")}
        wrest = {nm: cpool.tile([P, NHT * 512], bf16, tag=f"wr{nm}", name=f"wr{nm}")
                 for nm in ("q", "k")}
        wv_t = cpool.tile([P, NHT * GW], bf16, tag="wv")

        def h_ap(i, c0, c1):
            return hsb[i // 2][:, (i % 2) * S + c0:(i % 2) * S + c1]

        def w_ap(nm, i, c0, c1):
            if nm == "v":
                return wv_t[:, i * GW + c0:i * GW + c1]
            if c1 <= P:
                return wj0[nm][:, i * P + c0:i * P + c1]
            return wrest[nm][:, i * 512 + (c0 - P):i * 512 + (c1 - P)]

        if not skip_in:
            nc.sync.dma_start(
                wj0["q"][:].rearrange("p (i c) -> p i c", c=P),
                w_r["q"][:, :, 0:P])
            for i2 in range(NHT // 2):
                nc.sync.dma_start(hsb[i2][:], hT_r[:, 2 * i2:2 * i2 + 2])
            nc.sync.dma_start(
                wj0["k"][:].rearrange("p (i c) -> p i c", c=P),
                w_r["k"][:, :, 0:P])
            nc.sync.dma_start(rot_t[:], rotb)
            for nm in ("q", "k"):
                nc.sync.dma_start(
                    wrest[nm][:].rearrange("p (i c) -> p i c", c=512),
                    w_r[nm][:, :, P:GW])
            nc.sync.dma_start(wv_t[:], w_r["v"])

        # V in natural layout with a ones column appended per head (gives the
        # softmax denominator in the PV matmul).
        vsb = []
        for st in range(NKT):
            t = cpool.tile([P, NHC * 65], bf16, tag=f"v{st}")
            ones_ap = t[:].rearrange("p (h c) -> p h c", c=65)[:, :, 64:65]
            nc.vector.memset(ones_ap, 1.0)
            vsb.append(t)

        def emit_vchunk(st, n0, n1):
            vps = pspool.tile([P, 512], f32, tag="proj")
            for i in range(NHT):
                nc.tensor.matmul(vps[:, 0:n1 - n0],
                                 lhsT=h_ap(i, st * P, (st + 1) * P),
                                 rhs=w_ap("v", i, n0, n1),
                                 start=(i == 0), stop=(i == NHT - 1))
            h0 = n0 // 64
            dst = vsb[st][:, h0 * 65:(n1 // 64) * 65] \
                .rearrange("p (h c) -> p h c", c=65)[:, :, 0:64]
            src = vps[:, 0:n1 - n0].rearrange("p (h c) -> p h c", c=64)
            nc.vector.tensor_copy(dst, src)

        def proj_chunks(j, qk_out):
            """Q/K projection + rotary for pair j as 6 callbacks:
            (q-half0, q-half1, q-finish, k-half0, k-half1, k-finish).
            The finishers store the fin tiles into qk_out."""
            chunks = []
            hold = {}
            for nm in ("q", "k"):
                def halfchunk(nm=nm, half=None):
                    if half == 0:
                        hold[nm] = stpool.tile([P, S], bf16, tag=f"{nm}ps",
                                               name=f"{nm}ps")
                    qps = hold[nm]
                    c0, c1 = half * 512, (half + 1) * 512
                    ps = pspool.tile([P, 512], f32, tag="proj")
                    for i in range(NHT):
                        nc.tensor.matmul(ps[:],
                                         lhsT=w_ap(nm, i, j * P, (j + 1) * P),
                                         rhs=h_ap(i, c0, c1),
                                         start=(i == 0), stop=(i == NHT - 1))
                    nc.vector.tensor_copy(qps[:, c0:c1], ps[:])

                def finish(nm=nm):
                    qps = hold[nm]
                    if "rot" in ablate:
                        qk_out[nm] = qps
                        return
                    qsh = stpool.tile([P, S], bf16, tag=f"{nm}sh", name=f"{nm}sh")
                    for d0, s0 in ((0, 32), (32, 0), (64, 96), (96, 64)):
                        nc.sync.dma_start(qsh[d0:d0 + 32, :], qps[s0:s0 + 32, :])
                    t1 = stpool.tile([P, S], bf16, tag=f"{nm}t1", name=f"{nm}t1")
                    nc.vector.tensor_mul(t1[:], qps[:], cos_t[:])
                    nc.vector.tensor_mul(qsh[:], qsh[:], ssin_t[:])
                    fin = qkpool.tile([P, S], bf16, tag=nm, name=nm)
                    nc.vector.tensor_add(fin[:], t1[:], qsh[:])
                    qk_out[nm] = fin
                chunks.append(lambda f=halfchunk: f(half=0))
                chunks.append(lambda f=halfchunk: f(half=1))
                chunks.append(finish)
            return chunks

        def emit_score_tile(j, qk, probs, sub, kt):
            sps = sspool.tile([P, S], f32, tag="sc")
            for qh in (0, 1):
                nc.tensor.matmul(
                    sps[:, qh * 512:(qh + 1) * 512],
                    lhsT=qk["k"][sub * 64:(sub + 1) * 64, kt * P:(kt + 1) * P],
                    rhs=qk["q"][sub * 64:(sub + 1) * 64, qh * 512:(qh + 1) * 512],
                    start=True, stop=True)
            pr = ppool.tile([P, S], bf16, tag=f"pr{sub}{kt}", name=f"pr{sub}{kt}")
            if "exp" in ablate:
                nc.vector.tensor_copy(pr[:, 0:4], sps[:, 0:4])
            else:
                nc.scalar.activation(pr[:], sps[:], Exp)
            probs[sub, kt] = pr

        def emit_scores(j, qk, pending):
            """Scores+exp for pair j; `pending` callbacks fill PE gaps."""
            it = iter(pending)
            probs = {}
            for kt in range(NKT):
                for sub in (0, 1):
                    c = next(it, None)
                    if c is not None:
                        c()
                    emit_score_tile(j, qk, probs, sub, kt)
            for c in it:
                c()
            return probs

        def pv_chunks(j, probs, subs=(0, 1)):
            """PV for pair j: per (sub, qhalf) bank, two callbacks each."""
            if probs is None or "pv" in ablate:
                return []
            chunks = []
            holder = {}
            for sub in subs:
                hl = 2 * j + sub
                for qh in (0, 1):
                    def bank_mm(sub=sub, hl=hl, qh=qh, qls=(0, 1)):
                        if qls[0] == 0:
                            holder[sub, qh] = scpool.tile([P, 512], f32, tag="ctx", name="cps")
                        cps = holder[sub, qh]
                        for ql in qls:
                            qt = qh * 4 + ql
                            for kt in range(NKT):
                                nc.tensor.matmul(
                                    cps[:, ql * 65:(ql + 1) * 65],
                                    lhsT=probs[sub, kt][:, qt * P:(qt + 1) * P],
                                    rhs=vsb[kt][:, hl * 65:(hl + 1) * 65],
                                    start=(ql == 0 and kt == 0),
                                    stop=(ql == 3 and kt == NKT - 1),
                                    skip_group_check=True)

                    def bank_fin(sub=sub, hl=hl, qh=qh):
                        bank_mm(sub, hl, qh, qls=(2, 3))
                        cps = holder[sub, qh]
                        c3 = cps[:, 0:260].rearrange("p (q c) -> p q c", c=65)
                        rcp = opool.tile([P, 4], f32, tag="rcp")
                        r3 = rcp[:].rearrange("p (q o) -> p q o", o=1)
                        nc.vector.reciprocal(r3, c3[:, :, 64:65])
                        if qh == 0:
                            osb = opool.tile([P, 512], bf16, tag="osb")
                            holder["o", sub] = osb
                        osb = holder["o", sub]
                        o3 = osb[:, qh * 256:(qh + 1) * 256] \
                            .rearrange("p (q c) -> p q c", c=64)
                        nc.vector.tensor_mul(o3, c3[:, :, 0:64],
                                             r3.broadcast_to([P, 4, 64]))
                        if qh == 1:
                            dst = out[:, hl * 64:(hl + 1) * 64] \
                                .rearrange("(t p) c -> p t c", p=P)
                            src = osb[:].rearrange("p (t c) -> p t c", c=64)
                            nc.sync.dma_start(dst, src)
                    chunks.append(bank_mm)
                    chunks.append(bank_fin)
            return chunks

        def emit_scores_last(j, qk, pending):
            """Last pair: sub-major scores so sub0's PV can interleave into
            sub1's exp-gated window; sub1's PV is the (short) tail."""
            from itertools import chain as _chain
            it = iter(pending)
            probs = {}
            for sub in (0, 1):
                for kt in range(NKT):
                    c = next(it, None)
                    if c is not None:
                        c()
                    emit_score_tile(j, qk, probs, sub, kt)
                if sub == 0:
                    it = _chain(it, pv_chunks(j, probs, subs=(0,)))
            for c in it:
                c()
            for c in pv_chunks(j, probs, subs=(1,)):
                c()
            return probs

        def interleave(a, b):
            o, ia, ib = [], iter(a), iter(b)
            while True:
                x = next(ia, None)
                y = next(ib, None)
                if x is None and y is None:
                    return o
                o += ([x] if x is not None else []) + ([y] if y is not None else [])

        # Software pipeline: pair j+1's projection+rotary, pair j-1's PV and
        # (at j=0/1) the V projection all run as PE filler chunks inside the
        # exp-gated scores loops.
        qk = {}
        for c in proj_chunks(0, qk):
            c()
        vjobs = [(lambda st=st, n0=n0, n1=n1: emit_vchunk(st, n0, n1))
                 for st in range(NKT) for n0, n1 in ((0, 512), (512, GW))]
        prev_pv = []
        for j in range(NPAIR):
            qk_next = {}
            pj = proj_chunks(j + 1, qk_next) if j + 1 < NPAIR else []
            if j == 0:
                # v-chunks must trail the wv load; PV(0) (emitted in pair 1)
                # reads all of vsb, so st 4..7 lead pair 1's list.
                pending = pj + vjobs[:8]
            elif j == 1:
                pending = interleave(vjobs[8:], pj) + prev_pv
            elif j == NPAIR - 2:
                # hold back part of PV(j-1) to feed the last pair's first half
                pending = interleave(prev_pv[:6], pj)
                spill = prev_pv[6:]
            elif j == NPAIR - 1:
                pending = spill + prev_pv
            else:
                pending = interleave(prev_pv, pj)
            if j == NPAIR - 1:
                emit_scores_last(j, qk, pending)
            else:
                probs = emit_scores(j, qk, pending)
                prev_pv = pv_chunks(j, probs)
                qk = qk_next

    nc.compile()
    return nc


def _host_prep(hidden_states, Wq, Wk, Wv, np_dt=None):
    bf = ml_dtypes.bfloat16
    scale = np.float32(HD ** -0.5)
    inv_freq = 1.0 / (10000.0 ** (np.arange(0, HD, 2) / HD))
    emb = np.concatenate([np.outer(np.arange(S), inv_freq)] * 2, 1)  # [S, 64]
    cosT = np.cos(emb).T.astype(np.float32)                          # [64, S]
    sign = np.where(np.arange(HD) < 32, -1.0, 1.0).astype(np.float32)
    ssinT = (np.sin(emb).astype(np.float32) * sign).T
    rotb = np.ascontiguousarray(np.concatenate(
        [np.concatenate([cosT, cosT], 0),
         np.concatenate([ssinT, ssinT], 0)], 1)).astype(bf)          # [128, 2S]

    in_maps = []
    for c in range(N_CORES):
        b, g = c // 2, c % 2
        sl = slice(g * GW, (g + 1) * GW)
        in_maps.append({
            "hT": np.ascontiguousarray(np.asarray(hidden_states[b]).T).astype(bf),
            "wqT": np.ascontiguousarray((Wq[sl] * scale).T).astype(bf),
            "wkT": np.ascontiguousarray(Wk[sl].T).astype(bf),
            "wvT": np.ascontiguousarray(Wv[sl].T).astype(bf),
            "rotb": rotb,
        })
    return in_maps


def get_compiled(dt_name=None, loop_reps=1, ablate=(), pv_mode=None):
    key = (loop_reps, tuple(sorted(ablate)))
    if key not in _cache:
        _cache[key] = _build(loop_reps, ablate)
    return _cache[key], "bf16"


def run(inputs, trace=False, dt_name=None):
    """Returns (full_output, BassKernelResults)."""
    from concourse import bass_utils
    nc, _ = get_compiled()
    in_maps = _host_prep(np.asarray(inputs["hidden_states"]),
                         np.asarray(inputs["Wq"]), np.asarray(inputs["Wk"]),
                         np.asarray(inputs["Wv"]))
    res = bass_utils.run_bass_kernel_spmd(nc, in_maps,
                                          core_ids=list(range(N_CORES)),
                                          trace=trace)
    full = np.zeros((B, S, H), np.float32)
    for c in range(N_CORES):
        b, g = c // 2, c % 2
        full[b, :, g * GW:(g + 1) * GW] = res.results[c]["out"].astype(np.float32)
    return full, res


def kernel(**inputs):
    full, _ = run(inputs)
    return full
